# revision 44
# baseline (speedup 1.0000x reference)
"""Trainium2 Bass kernel for nn_Decoder_29678224015654.

Architecture (hardcoded from spec):
  SEQ=32, SUB=16, B=512, V=258, U=2, Z=512, CH=CO... CH=1024, CO=512, DH=1024.

Sharding: data-parallel over batch. 8 cores x 64 batch rows. Each core runs
the conductor (eff batch 64) and the decoder with both U=2 segments batched
along the free axis (eff batch 128). Weights replicated; no collectives.

Layout: feature-major. Activations live in SBUF as [128 partitions = feature
slice, batch in free dim]. All matmuls: lhsT = weight [K-chunk(128), M-tile
(128 gate cols)] bf16 stationary; rhs = activation [K-chunk, batch] bf16
moving; accumulate f32 in PSUM. Gate nonlinearities on ScalarE directly from
PSUM with per-partition bias; cell math on VectorE in f32.
"""
import sys
sys.path.insert(0, '/opt/trn_rl_repo')

import numpy as np
import ml_dtypes

import concourse.bass as bass
import concourse.mybir as mybir
from concourse import bacc
from concourse.tile import TileContext
from concourse.bass_utils import run_bass_kernel_spmd

AF = mybir.ActivationFunctionType
BF = mybir.dt.bfloat16
F32 = mybir.dt.float32

SEQ, SUB, B, V = 32, 16, 512, 258
U = SEQ // SUB              # 2
Z, CH, CO, DH = 512, 1024, 512, 1024
NCORES = 8
BL = B // NCORES            # 64 batch rows per core
BD = BL * U                 # 128 decoder effective batch (both segments)
VP = 384                    # V padded to 3 K-chunks
G = 4 * DH                  # 4096 gate cols (same for conductor: 4*CH)
NT = G // 128               # 32 gate tiles
KH = DH // 128              # 8 K-chunks for hidden=1024
KZ = Z // 128               # 4 K-chunks for 512
KV = VP // 128              # 3 K-chunks for padded V

bf16 = ml_dtypes.bfloat16


# --------------------------------------------------------------------------
# Bass program
# --------------------------------------------------------------------------

def build_nc(nsteps_cond=U, nsteps_dec=SUB, debug_outs=False):
    nc = bacc.Bacc("TRN2", target_bir_lowering=False, debug=False,
                   num_devices=NCORES)

    def inp(name, shape, dtype=BF):
        return nc.dram_tensor(name, list(shape), dtype, kind="ExternalInput")

    # m-tile-major weights: [m, 128, Kin] ; lhsT slice (m, k) = [:, m*Kin+k*128]
    d_w1h = inp("w1h", [NT, 128, DH])          # resident
    d_w2i = inp("w2i", [NT, 128, DH])          # streamed
    d_w2h = inp("w2h", [NT, 128, DH])          # streamed
    d_w1i = inp("w1i", [NT, 128, VP])          # streamed (prev-token part)
    d_w1c = inp("w1c", [NT, 128, CO])          # resident (c part of dW1i)
    d_cw1h = inp("cw1h", [NT, 128, CH])        # streamed (conductor)
    d_cw2i = inp("cw2i", [NT, 128, CH])        # streamed
    d_cw2h = inp("cw2h", [NT, 128, CH])        # streamed
    d_cw1i = inp("cw1i", [NT, 128, VP])        # resident (conductor input part)
    d_ciw = inp("ciw", [KH, 128, Z])           # h1-init weight [8 m-tiles,128,512]
    d_cow = inp("cow", [KZ, 128, CH])          # c head [4 m-tiles, 128, 1024]
    d_diw = inp("diw", [KH, 128, CO])          # decoder h1_0 weight
    d_fcw = inp("fcw", [KH, 128, V])           # fc, k-chunk-major (moving)

    d_id = inp("ident", [128, 128])            # bf16 identity
    d_db1 = inp("db1", [128, NT], F32)         # db1i+db1h packed [128,32]
    d_db2 = inp("db2", [128, NT], F32)
    d_cb1 = inp("cb1", [128, NT], F32)
    d_cb2 = inp("cb2", [128, NT], F32)
    d_cib = inp("cib", [128, KH], F32)
    d_dib = inp("dib", [128, KH], F32)
    d_cob = inp("cob", [128, KZ], F32)
    d_fcb = inp("fcb", [1, V])                 # bf16
    d_ones = inp("ones", [1, 128])             # bf16 ones

    d_zt = inp("zt", [KZ, 128, BL])            # z^T per core, bf16
    d_cin = inp("cin", [KV, 128, BL])          # conductor_input^T padded, bf16
    d_prev = inp("prev", [SUB, 128, KV * BD])  # prev tokens [t][p][c*128+ub]

    d_out = nc.dram_tensor("out", [BL, SEQ, V], F32, kind="ExternalOutput")
    dbg = {}
    if debug_outs:
        for nm, w in (("dbg_h1d", KH * BD), ("dbg_h2d", KH * BD),
                      ("dbg_c2d", KH * BD), ("dbg_ct", KZ * BD)):
            dbg[nm] = nc.dram_tensor(nm, [128, w], F32, kind="ExternalOutput")

    with TileContext(nc) as tc:
        with (
            tc.sbuf_pool(name="const", bufs=1) as cp,
            tc.sbuf_pool(name="work", bufs=3) as wk,
            tc.psum_pool(name="pg", bufs=7) as pg,
            tc.psum_pool(name="pfc", bufs=1) as pfc,
        ):
            # ---------------- resident loads ----------------
            def load_const(name, dram, shape, dtype=BF):
                t = cp.tile(shape, dtype, tag=name)
                nc.sync.dma_start(out=t[:], in_=dram[:])
                return t

            # resident weights
            w1h = cp.tile([128, NT * DH], BF, tag="w1h")
            for m in range(NT):
                nc.scalar.dma_start(out=w1h[:, m * DH:(m + 1) * DH], in_=d_w1h[m])
            NRES = 12   # first 12 m-tiles of w2h resident
            w2hr = cp.tile([128, NRES * DH], BF, tag="w2hr")
            for m in range(NRES):
                nc.scalar.dma_start(out=w2hr[:, m * DH:(m + 1) * DH], in_=d_w2h[m])
            fcw = cp.tile([128, KH * V], BF, tag="fcw")
            for k in range(KH):
                nc.sync.dma_start(out=fcw[:, k * V:(k + 1) * V], in_=d_fcw[k])

            ident = load_const("ident", d_id, [128, 128], BF)
            db1 = load_const("db1", d_db1, [128, NT], F32)
            db2 = load_const("db2", d_db2, [128, NT], F32)
            cb1 = load_const("cb1", d_cb1, [128, NT], F32)
            cb2 = load_const("cb2", d_cb2, [128, NT], F32)
            cib = load_const("cib", d_cib, [128, KH], F32)
            dib = load_const("dib", d_dib, [128, KH], F32)
            cob = load_const("cob", d_cob, [128, KZ], F32)
            fcb = load_const("fcb", d_fcb, [1, V], BF)
            ones = load_const("ones", d_ones, [1, 128], BF)

            zt = cp.tile([128, KZ * BL], BF, tag="zt")
            for k in range(KZ):
                nc.sync.dma_start(out=zt[:, k * BL:(k + 1) * BL], in_=d_zt[k])
            cin = cp.tile([128, KV * BL], BF, tag="cin")
            for k in range(KV):
                nc.sync.dma_start(out=cin[:, k * BL:(k + 1) * BL], in_=d_cin[k])

            # ---------------- state arrays ----------------
            def state(name, w, dtype=F32, pool=cp):
                return pool.tile([128, w], dtype, tag=name, name=name)

            h1d = state("h1d", KH * BD); c1d = state("c1d", KH * BD)
            h2d = state("h2d", KH * BD); c2d = state("c2d", KH * BD)
            h1db = state("h1db", KH * BD, BF); h2db = state("h2db", KH * BD, BF)
            ctb = state("ctb", KZ * BD, BF)    # c^T both segments, bf16
            zdec = state("zdec", NT * BD, BF)  # dW1i_c @ c term, bf16
            # gate arrays (f32) - shared conductor/decoder, sized for decoder
            gi = state("gi", KH * BD); gf = state("gf", KH * BD)
            gg = state("gg", KH * BD); go = state("go", KH * BD)
            tmp = state("tmp", KH * BD)

            # ---------------- helpers ----------------
            def lstm_layer(bl, m_w_rhs, bias, cst, hst, hbf, zadd=None):
                """One LSTM layer update for all NT gate tiles.

                m_w_rhs(m) -> list of (lhsT_ap, rhs_ap) matmul pairs.
                cst/hst f32 state [128, KH*bl]; hbf bf16 copy of hst.
                zadd: optional [128, NT*bl] step-constant gate pre-activation
                term added on VectorE (saves a PE identity matmul per tile).
                """
                for m in range(NT):
                    ps = pg.tile([128, bl], F32, tag="ps")
                    pairs = m_w_rhs(m)
                    for j, (lh, rh) in enumerate(pairs):
                        nc.tensor.matmul(ps[:], lh, rh, start=(j == 0),
                                         stop=(j == len(pairs) - 1))
                    ty, sub = divmod(m, KH)
                    func = AF.Tanh if ty == 2 else AF.Sigmoid
                    dst = (gi, gf, gg, go)[ty][:, sub * bl:(sub + 1) * bl]
                    if zadd is not None:
                        pre = wk.tile([128, bl], F32, tag="pre")
                        nc.vector.tensor_add(pre[:], ps[:],
                                             zadd[:, m * bl:(m + 1) * bl])
                        nc.scalar.activation(dst, pre[:], func,
                                             bias=bias[:, m:m + 1])
                    else:
                        nc.scalar.activation(dst, ps[:], func,
                                             bias=bias[:, m:m + 1])
                w = KH * bl
                # c = gf*c + gi*gg ; h = go*tanh(c)
                # chunked so early h slices unblock the next layer's matmuls
                cw = w // 4
                for q in range(4):
                    s = slice(q * cw, (q + 1) * cw)
                    nc.vector.tensor_mul(tmp[:, s], gi[:, s], gg[:, s])
                    nc.vector.tensor_mul(cst[:, s], gf[:, s], cst[:, s])
                    nc.vector.tensor_add(cst[:, s], cst[:, s], tmp[:, s])
                    nc.scalar.activation(tmp[:, s], cst[:, s], AF.Tanh)
                    nc.vector.tensor_mul(hst[:, s], go[:, s], tmp[:, s])
                    nc.vector.tensor_copy(hbf[:, s], hst[:, s])

            # ================= conductor (scoped pool) =================
            # Conductor state (width KH*BL=512) aliases the first half of the
            # decoder state arrays, which are dead until the prologue ends.
            # Decoder memsets therefore happen AFTER the conductor.
            with tc.sbuf_pool(name="cond", bufs=1) as cpd:
                h1c, c1c, h2c, c2c = h1d, c1d, h2d, c2d
                h1cb, h2cb = h1db, h2db
                zcon = zdec
                for t in (c1c, h2c, c2c, h2cb):
                    nc.vector.memset(t[:, :KH * BL], 0.0)

                # conductor input term: zcon[m] = sum_k cw1i[m,k]^T @ cin[k]
                for m in range(NT):
                    w = cpd.tile([128, VP], BF, tag="s_cw1i", bufs=3)
                    eng = nc.gpsimd if m % 2 else nc.sync
                    eng.dma_start(out=w[:], in_=d_cw1i[m])
                    ps = pg.tile([128, BL], F32, tag="ps")
                    for k in range(KV):
                        nc.tensor.matmul(
                            ps[:], w[:, k * 128:(k + 1) * 128],
                            cin[:, k * BL:(k + 1) * BL],
                            start=(k == 0), stop=(k == KV - 1))
                    nc.scalar.activation(zcon[:, m * BL:(m + 1) * BL], ps[:],
                                         AF.Copy)

                # h1c[m] = tanh(sum_k ciw[m,k]^T @ zt[k] + cib[m])
                for m in range(KH):
                    w = cpd.tile([128, Z], BF, tag="s_ciw", bufs=3)
                    nc.gpsimd.dma_start(out=w[:], in_=d_ciw[m])
                    ps = pg.tile([128, BL], F32, tag="ps")
                    for k in range(KZ):
                        nc.tensor.matmul(
                            ps[:], w[:, k * 128:(k + 1) * 128],
                            zt[:, k * BL:(k + 1) * BL],
                            start=(k == 0), stop=(k == KZ - 1))
                    nc.scalar.activation(h1c[:, m * BL:(m + 1) * BL], ps[:],
                                         AF.Tanh, bias=cib[:, m:m + 1])
                nc.vector.tensor_copy(h1cb[:], h1c[:])

                for step in range(nsteps_cond):
                    def l1(m):
                        w = cpd.tile([128, CH], BF, tag="s_cw1h", bufs=6)
                        eng = nc.gpsimd if m % 2 else nc.sync
                        eng.dma_start(out=w[:], in_=d_cw1h[m])
                        prs = [(w[:, k * 128:(k + 1) * 128],
                                h1cb[:, k * BL:(k + 1) * BL])
                               for k in range(KH)]
                        return prs
                    lstm_layer(BL, l1, cb1, c1c, h1c, h1cb, zadd=zcon)

                    def l2(m):
                        wi = cpd.tile([128, CH], BF, tag="s_cw2i", bufs=6)
                        wh = cpd.tile([128, CH], BF, tag="s_cw2h", bufs=6)
                        nc.sync.dma_start(out=wi[:], in_=d_cw2i[m])
                        nc.gpsimd.dma_start(out=wh[:], in_=d_cw2h[m])
                        # h2 (old state) first: no wait on layer-1 cell math
                        prs = [(wh[:, k * 128:(k + 1) * 128],
                                h2cb[:, k * BL:(k + 1) * BL])
                               for k in range(KH)]
                        prs += [(wi[:, k * 128:(k + 1) * 128],
                                 h1cb[:, k * BL:(k + 1) * BL])
                                for k in range(KH)]
                        return prs
                    lstm_layer(BL, l2, cb2, c2c, h2c, h2cb)

                    # c^T[step] = cow^T @ h2c + cob -> ctb bf16
                    for m in range(KZ):
                        w = cpd.tile([128, CH], BF, tag="s_cow", bufs=3)
                        nc.gpsimd.dma_start(out=w[:], in_=d_cow[m])
                        ps = pg.tile([128, BL], F32, tag="ps")
                        for k in range(KH):
                            nc.tensor.matmul(
                                ps[:], w[:, k * 128:(k + 1) * 128],
                                h2cb[:, k * BL:(k + 1) * BL],
                                start=(k == 0), stop=(k == KH - 1))
                        nc.scalar.activation(
                            ctb[:, m * BD + step * BL: m * BD + (step + 1) * BL],
                            ps[:], AF.Identity, bias=cob[:, m:m + 1])

                # decoder h1_0 = tanh(diw^T @ ct + dib)
                for m in range(KH):
                    w = cpd.tile([128, CO], BF, tag="s_diw", bufs=2)
                    nc.gpsimd.dma_start(out=w[:], in_=d_diw[m])
                    ps = pg.tile([128, BD], F32, tag="ps")
                    for k in range(KZ):
                        nc.tensor.matmul(
                            ps[:], w[:, k * 128:(k + 1) * 128],
                            ctb[:, k * BD:(k + 1) * BD],
                            start=(k == 0), stop=(k == KZ - 1))
                    nc.scalar.activation(h1d[:, m * BD:(m + 1) * BD], ps[:],
                                         AF.Tanh, bias=dib[:, m:m + 1])
                nc.vector.tensor_copy(h1db[:], h1d[:])

                # decoder c-term: zdec[m] = sum_k w1c[m,k]^T @ ct[k]
                for m in range(NT):
                    w = cpd.tile([128, CO], BF, tag="s_w1c", bufs=3)
                    eng = nc.gpsimd if m % 2 else nc.sync
                    eng.dma_start(out=w[:], in_=d_w1c[m])
                    ps = pg.tile([128, BD], F32, tag="ps")
                    for k in range(KZ):
                        nc.tensor.matmul(
                            ps[:], w[:, k * 128:(k + 1) * 128],
                            ctb[:, k * BD:(k + 1) * BD],
                            start=(k == 0), stop=(k == KZ - 1))
                    nc.scalar.activation(zdec[:, m * BD:(m + 1) * BD], ps[:],
                                         AF.Copy)

            # decoder zero-init (after conductor: state arrays were aliased)
            for t in (c1d, h2d, c2d, h2db):
                nc.vector.memset(t[:], 0.0)

            # ================= decoder steps =================
            with tc.sbuf_pool(name="dstream", bufs=8) as wp:
                for step in range(nsteps_dec):
                    prev = wk.tile([128, KV * BD], BF, tag="prev")
                    nc.gpsimd.dma_start(out=prev[:], in_=d_prev[step])

                    def l1(m, _p=prev):
                        w = wp.tile([128, VP], BF, tag="w1i")
                        nc.gpsimd.dma_start(out=w[:], in_=d_w1i[m])
                        prs = [(w[:, k * 128:(k + 1) * 128],
                                _p[:, k * BD:(k + 1) * BD]) for k in range(KV)]
                        prs += [(w1h[:, m * DH + k * 128: m * DH + (k + 1) * 128],
                                 h1db[:, k * BD:(k + 1) * BD])
                                for k in range(KH)]
                        return prs
                    lstm_layer(BD, l1, db1, c1d, h1d, h1db, zadd=zdec)

                    def l2(m):
                        wi = wp.tile([128, DH], BF, tag="w2i")
                        nc.sync.dma_start(out=wi[:], in_=d_w2i[m])
                        if m < NRES:
                            def whs(k, _m=m):
                                return w2hr[:, _m * DH + k * 128:
                                            _m * DH + (k + 1) * 128]
                        else:
                            wht = wp.tile([128, DH], BF, tag="w2h")
                            nc.gpsimd.dma_start(out=wht[:], in_=d_w2h[m])
                            def whs(k, _w=wht):
                                return _w[:, k * 128:(k + 1) * 128]
                        # h2 (old state) first: no wait on layer-1 cell math
                        prs = [(whs(k),
                                h2db[:, k * BD:(k + 1) * BD])
                               for k in range(KH)]
                        prs += [(wi[:, k * 128:(k + 1) * 128],
                                 h1db[:, k * BD:(k + 1) * BD])
                                for k in range(KH)]
                        return prs
                    lstm_layer(BD, l2, db2, c2d, h2d, h2db)

                    # fc + log_softmax: psum[b, v] = h2^T fc_W + fc_b
                    ps = pfc.tile([128, V], F32, tag="pfc")
                    nc.tensor.matmul(ps[:], ones[:, :128], fcb[:], start=True,
                                     stop=False)
                    for k in range(KH):
                        nc.tensor.matmul(ps[:], h2db[:, k * BD:(k + 1) * BD],
                                         fcw[:, k * V:(k + 1) * V],
                                         start=False, stop=(k == KH - 1))
                    nmx = wk.tile([128, 1], F32, tag="nmx")
                    nc.vector.reduce_max(nmx[:], ps[:],
                                         axis=mybir.AxisListType.X,
                                         negate=True)
                    ex = wk.tile([128, V], F32, tag="ex")
                    se = wk.tile([128, 1], F32, tag="se")
                    nc.scalar.activation(ex[:], ps[:], AF.Exp, bias=nmx[:],
                                         accum_out=se[:])
                    lse = wk.tile([128, 1], F32, tag="lse")
                    nc.scalar.activation(lse[:], se[:], AF.Ln)
                    nc.vector.tensor_sub(lse[:], lse[:], nmx[:])
                    ot = wk.tile([128, V], F32, tag="ot")
                    nc.vector.tensor_scalar_sub(ot[:], ps[:], lse[:])
                    for u in range(U):
                        nc.sync.dma_start(
                            out=d_out[0:BL, u * SUB + step, :],
                            in_=ot[u * BL:(u + 1) * BL, :])

            if debug_outs:
                nc.sync.dma_start(out=dbg["dbg_h1d"][:], in_=h1d[:])
                nc.sync.dma_start(out=dbg["dbg_h2d"][:], in_=h2d[:])
                nc.sync.dma_start(out=dbg["dbg_c2d"][:], in_=c2d[:])
                ctf = cp.tile([128, KZ * BD], F32, tag="ctf")
                nc.vector.tensor_copy(ctf[:], ctb[:])
                nc.sync.dma_start(out=dbg["dbg_ct"][:], in_=ctf[:])

    nc.compile()
    return nc


# --------------------------------------------------------------------------
# Host-side packing
# --------------------------------------------------------------------------

def _mmajor(w, kin_pad=None):
    """[Kin, G] f32 -> [G//128, 128, Kin_pad] bf16, m-tile-major lhsT chunks.

    dst[m, p, k*128 + j] = w[k*128 + p, m*128 + j]  (partition = K row).
    """
    kin, g = w.shape
    kp = kin_pad or kin
    nt = g // 128
    wp = np.zeros((kp, g), dtype=np.float32)
    wp[:kin] = w
    out = (wp.reshape(kp // 128, 128, nt, 128)
             .transpose(2, 1, 0, 3)
             .reshape(nt, 128, kp))
    return np.ascontiguousarray(out.astype(bf16))


def _kpack(b):
    """[G] f32 -> [128, G//128] f32 column-per-tile bias pack."""
    return np.ascontiguousarray(b.reshape(-1, 128).T.astype(np.float32))


def pack_inputs(i):
    """Build the static (batch-independent) input map pieces."""
    m = {}
    m["w1h"] = _mmajor(i["dW1h"])
    m["w2i"] = _mmajor(i["dW2i"])
    m["w2h"] = _mmajor(i["dW2h"])
    m["w1i"] = _mmajor(i["dW1i"][CO:], VP)       # prev part, pad 258->384
    m["w1c"] = _mmajor(i["dW1i"][:CO])           # c part
    m["cw1h"] = _mmajor(i["cW1h"])
    m["cw2i"] = _mmajor(i["cW2i"])
    m["cw2h"] = _mmajor(i["cW2h"])
    m["cw1i"] = _mmajor(i["cW1i"], VP)
    m["ciw"] = _mmajor(i["ci_W"])                # [8,128,512]
    m["cow"] = _mmajor(i["co_W"])                # [4,128,1024]
    m["diw"] = _mmajor(i["di_W"])                # [8,128,512]
    # fc: k-chunk-major moving operand [KH,128,V]
    m["fcw"] = np.ascontiguousarray(
        i["fc_W"].reshape(KH, 128, V)).astype(bf16)
    m["db1"] = _kpack(i["db1i"] + i["db1h"])
    m["db2"] = _kpack(i["db2i"] + i["db2h"])
    m["cb1"] = _kpack(i["cb1i"] + i["cb1h"])
    m["cb2"] = _kpack(i["cb2i"] + i["cb2h"])
    m["cib"] = _kpack(i["ci_b"])
    m["dib"] = _kpack(i["di_b"])
    m["cob"] = _kpack(i["co_b"])
    m["fcb"] = i["fc_b"].reshape(1, V).astype(bf16)
    m["ones"] = np.ones((1, 128), dtype=bf16)
    m["ident"] = np.eye(128, dtype=bf16)
    return m


def pack_core(i, core):
    """Per-core batch-sharded inputs."""
    s = slice(core * BL, (core + 1) * BL)
    m = {}
    zt = i["z"][s].T.astype(bf16)                          # [512, BL]
    m["zt"] = np.ascontiguousarray(zt.reshape(KZ, 128, BL))
    ci = np.zeros((VP, BL), dtype=bf16)
    ci[:V] = i["conductor_input"][0, s].T.astype(bf16)
    m["cin"] = np.ascontiguousarray(ci.reshape(KV, 128, BL))
    # prev tokens: [SUB, 128, KV*BD]; free idx = c*BD + u*BL + b
    x = i["x"]                                             # [SEQ, B, V]
    prev = np.zeros((SUB, KV, 128, U, BL), dtype=bf16)
    for t in range(SUB):
        for u in range(U):
            sq = u * SUB + t
            if sq == 0:
                pv = np.zeros((BL, V), np.float32)
                pv[:, 0] = 1.0
            else:
                pv = x[sq - 1, s]
            prev[t, :, :, u, :] = np.pad(
                pv.T.astype(bf16), ((0, VP - V), (0, 0))).reshape(KV, 128, BL)
    m["prev"] = np.ascontiguousarray(
        prev.transpose(0, 1, 2, 3, 4).reshape(SUB, KV, 128, U * BL)
            .transpose(0, 2, 1, 3).reshape(SUB, 128, KV * BD))
    return m


_NC_CACHE = {}


def _get_nc(key=(U, SUB, False)):
    if key not in _NC_CACHE:
        _NC_CACHE[key] = build_nc(*key)
    return _NC_CACHE[key]


def kernel(**inputs):
    inputs = {k: np.asarray(v) for k, v in inputs.items()}
    nc = _get_nc()
    shared = pack_inputs(inputs)
    in_maps = [dict(shared, **pack_core(inputs, c)) for c in range(NCORES)]
    r = run_bass_kernel_spmd(nc, in_maps, core_ids=list(range(NCORES)))
    out = np.concatenate([r.results[c]["out"] for c in range(NCORES)], axis=0)
    return out.astype(np.float32)
